# revision 13
# baseline (speedup 1.0000x reference)
"""Trainium2 8-core kernel for nn_Attention_76347338653911.

External-attention ViT block with training-mode sync-BatchNorm:
  qv = x @ W_qv ; q,v per head
  attn = softmax((BN(q@k_extT)+bias)*scale) ; out = (attn @ BN(v)) @ W_proj + b_proj

Math restructure used here:
  - BN on scores: mean/beta shift cancels in softmax ->
      softmax(alpha_a[h]*s * scores + s*bias_p),  alpha_a = gamma*rsqrt(var_a)
  - BN on v folds into the projection:
      out = U @ (alpha_v (.) W_proj) + (c_v @ W_proj + b_proj),
      c_v = beta - mean_v*alpha_v,   U = softmax-attn @ v  (un-BN'd v)
  - so the only cross-core communication is a 48-float AllReduce of
    per-head (sum, sumsq) statistics of the scores and of v.
  - score stats without materializing scores:
      sum_attn[h]   = ksum . colsum(q_h)          (ksum = k_ext.sum(0))
      sumsq_attn[h] = || q_h @ L ||_F^2,  L = chol(k_ext^T k_ext)  (host)

Sharding: data-parallel over batch B=64 -> 8 per core.
"""

import os
import sys
import numpy as np

sys.path.insert(0, "/opt/trn_rl_repo")

import ml_dtypes

BF = ml_dtypes.bfloat16

# problem dims (hardcoded)
B, N, C, H, HD = 64, 196, 768, 12, 64
BL = B // 8            # batch per core
TOK = BL * N           # 1568 tokens per core
PC = 98                # p-chunk (196 = 2*98)
TCH = 392              # token free-chunk (1568 = 4*392)
SCALE = HD ** -0.5     # 0.125
BN_EPS = 1e-5
NA = float(B * N * N)        # attn BN count per head (global)
NV = float(B * N * HD)       # v BN count per head (global)

_NC_CACHE = {}


def _build_nc():
    import concourse.bass as bass
    import concourse.mybir as mybir
    import concourse.tile as tile
    from concourse import bacc

    f32 = mybir.dt.float32
    bf16 = mybir.dt.bfloat16
    AF = mybir.ActivationFunctionType
    OP = mybir.AluOpType

    nc = bacc.Bacc("TRN2", target_bir_lowering=False, debug=False, num_devices=8)

    # ---- DRAM parameters (per-core shard views) ----
    xT_d = nc.dram_tensor("xT", [C, TOK], bf16, kind="ExternalInput")
    wq_d = nc.dram_tensor("wq", [C, C], bf16, kind="ExternalInput")
    wv_d = nc.dram_tensor("wv", [C, C], bf16, kind="ExternalInput")
    wp_d = nc.dram_tensor("wp", [C, C], bf16, kind="ExternalInput")
    kT_d = nc.dram_tensor("kT", [128, N], bf16, kind="ExternalInput")
    L_d = nc.dram_tensor("L", [128, HD], bf16, kind="ExternalInput")
    slhsA_d = nc.dram_tensor("slhsA", [128, 2], f32, kind="ExternalInput")
    sbias_d = nc.dram_tensor("sbias", [PC, 2], f32, kind="ExternalInput")
    gamma_d = nc.dram_tensor("gamma", [1, H], f32, kind="ExternalInput")
    beta_d = nc.dram_tensor("beta", [1, H], f32, kind="ExternalInput")
    bproj_d = nc.dram_tensor("bproj", [1, C], f32, kind="ExternalInput")
    R_d = nc.dram_tensor("R", [H, C], f32, kind="ExternalInput")
    out_d = nc.dram_tensor("out", [TOK, C], f32, kind="ExternalOutput")

    with tile.TileContext(nc) as tc:
        with (
            tc.tile_pool(name="persist", bufs=1) as pp,
            tc.tile_pool(name="dram", bufs=1, space="DRAM") as dramp,
        ):
            # ---- persistent SBUF tensors ----
            xT = pp.tile([128, 6, TOK], bf16, tag="xT")
            wq = pp.tile([128, 6, C], bf16, tag="wq")
            wv = pp.tile([128, 6, C], bf16, tag="wv")
            wp = pp.tile([128, 6, C], bf16, tag="wp")
            weff = pp.tile([128, 6, C], bf16, tag="weff")
            kT = pp.tile([128, N], bf16, tag="kT")   # k^T duplicated in both halves
            Ls = pp.tile([128, HD], bf16, tag="Ls")  # L duplicated in both halves
            slhsA = pp.tile([128, 2], f32, tag="slhsA")
            sbias = pp.tile([PC, 2], f32, tag="sbias")
            gam = pp.tile([1, H], f32, tag="gam")
            bet = pp.tile([1, H], f32, tag="bet")
            bproj = pp.tile([1, C], f32, tag="bproj")
            Rs = pp.tile([H, C], f32, tag="Rs")
            qT = pp.tile([128, 6, TOK], bf16, tag="qT")       # q^T channels x tok
            vpr = pp.tile([PC, 16, H, HD + 1], bf16, tag="vpr")  # v natural + ones col
            U_T = pp.tile([128, 6, TOK], bf16, tag="U_T")
            qcol = pp.tile([128, 6], f32, tag="qcol")
            ysq = pp.tile([HD, H], f32, tag="ysq")
            AR = pp.tile([1, 48], f32, tag="AR")
            Sg = pp.tile([1, 48], f32, tag="Sg")
            ones98 = pp.tile([PC, 1], bf16, tag="ones98")
            ones64 = pp.tile([HD, 1], f32, tag="ones64")
            expscb = pp.tile([PC, H], f32, tag="expscb")
            avb = pp.tile([128, H], f32, tag="avb")
            avc = pp.tile([128, 6], f32, tag="avc")
            beffb = pp.tile([128, C], f32, tag="beffb")
            cvT = pp.tile([H, 1], f32, tag="cvT")
            prm = pp.tile([1, 12 * 12], f32, tag="prm")       # param scratch
            beffr = pp.tile([1, C], f32, tag="beffr")

            arin = dramp.tile([1, 48], f32)
            arout = dramp.tile([1, 48], f32)
            cvd = dramp.tile([H, 1], f32)

            # ---- loads ----
            nc.sync.dma_start(xT[:], xT_d.ap().rearrange("(o p) t -> p o t", p=128))
            nc.sync.dma_start(wq[:], wq_d.ap().rearrange("(o p) t -> p o t", p=128))
            nc.sync.dma_start(wv[:], wv_d.ap().rearrange("(o p) t -> p o t", p=128))
            nc.sync.dma_start(wp[:], wp_d.ap().rearrange("(o p) t -> p o t", p=128))
            nc.sync.dma_start(kT[:], kT_d.ap())
            nc.sync.dma_start(Ls[:], L_d.ap())
            nc.sync.dma_start(slhsA[:], slhsA_d.ap())
            nc.sync.dma_start(sbias[:], sbias_d.ap())
            nc.sync.dma_start(gam[:], gamma_d.ap())
            nc.sync.dma_start(bet[:], beta_d.ap())
            nc.sync.dma_start(bproj[:], bproj_d.ap())
            nc.sync.dma_start(Rs[:], R_d.ap())

            nc.gpsimd.memset(ones98[:], 1.0)
            nc.gpsimd.memset(ones64[:], 1.0)
            nc.gpsimd.memset(vpr[:, :, :, HD:HD + 1], 1.0)
            nc.gpsimd.memset(AR[:], 0.0)

            def r4(ap):  # [p, 1568] -> [p, 4, 392]
                return ap.rearrange("p (a b) -> p a b", a=4)

            def r8(ap):  # [p, 1568] -> [p, 8, 196]
                return ap.rearrange("p (a b) -> p a b", a=8)

            def r2(ap):  # [p, 768] -> [p, 2, 384]
                return ap.rearrange("p (a b) -> p a b", a=2)

            # ================= Phase Q: q^T = (x @ Wq)^T =================
            with tc.tile_pool(name="psq", bufs=2, space="PSUM") as qpool:
                for ht in range(6):
                    qp = qpool.tile([128, 4, 512], f32, tag="qp")
                    for ncc in range(4):
                        for kc in range(6):
                            nc.tensor.matmul(
                                qp[:, ncc, :TCH],
                                wq[:, kc, ht * 128:(ht + 1) * 128],
                                xT[:, kc, ncc * TCH:(ncc + 1) * TCH],
                                start=(kc == 0), stop=(kc == 5),
                            )
                    # copy to qT (bf16) + per-channel column sums (free)
                    nc.scalar.activation(
                        r4(qT[:, ht, :]), qp[:, :, :TCH],
                        AF.Identity, accum_out=qcol[:, ht:ht + 1],
                    )

            # ================= Phase V: v natural + v stats ==============
            with (
                tc.tile_pool(name="psv", bufs=2, space="PSUM") as vpool,
                tc.tile_pool(name="psacc", bufs=1, space="PSUM") as apool,
                tc.tile_pool(name="vsq", bufs=2) as vsqp_pool,
            ):
                vsump = apool.tile([1, 2, 512], f32, tag="vsum")
                vsqp = apool.tile([1, 2, 512], f32, tag="vsq")
                for t in range(16):
                    vp = vpool.tile([PC, 2, 512], f32, tag="vp")
                    for n2 in range(2):
                        for kc in range(6):
                            nc.tensor.matmul(
                                vp[:, n2, :384],
                                xT[:, kc, t * PC:(t + 1) * PC],
                                wv[:, kc, n2 * 384:(n2 + 1) * 384],
                                start=(kc == 0), stop=(kc == 5),
                            )
                    # repack [98, 768] -> vpr[:, t, h, 0:64]
                    nc.vector.tensor_copy(
                        vpr[:, t, :, 0:HD].rearrange("p (a h) d -> p a h d", a=2),
                        vp[:, :, :384].rearrange("p a (h d) -> p a h d", h=6),
                    )
                    # squares for sumsq_v
                    vsq = vsqp_pool.tile([PC, C], bf16, tag="vsq")
                    nc.scalar.activation(r2(vsq[:]), vp[:, :, :384], AF.Square)
                    for half in range(2):
                        nc.tensor.matmul(
                            vsqp[:, half, :384], ones98[:],
                            vsq[:, half * 384:(half + 1) * 384],
                            start=(t == 0), stop=(t == 15),
                        )
                        nc.tensor.matmul(
                            vsump[:, half, :390], ones98[:],
                            vpr[:, t, 6 * half:6 * half + 6, :],
                            start=(t == 0), stop=(t == 15),
                        )
                # head-group the per-channel sums -> AR[24:36], AR[36:48]
                nc.vector.tensor_reduce(
                    AR[0:1, 24:36],
                    vsump[:, :, :390].rearrange("p a (h d) -> p a h d", h=6)[:, :, :, 0:HD],
                    axis=mybir.AxisListType.X, op=OP.add,
                )
                nc.vector.tensor_reduce(
                    AR[0:1, 36:48],
                    vsqp[:, :, :384].rearrange("p a (h d) -> p a h d", h=6),
                    axis=mybir.AxisListType.X, op=OP.add,
                )

            # ================= Phase Y: y = q @ L, sumsq_attn ============
            with (
                tc.tile_pool(name="psy", bufs=2, space="PSUM") as ypool,
                tc.tile_pool(name="ysqs", bufs=2) as ysq_pool,
            ):
                for h in range(H):
                    qb = (h % 2) * 64
                    yp = ypool.tile([HD, 4, 512], f32, tag="yp")
                    for ncc in range(4):
                        nc.tensor.matmul(
                            yp[:, ncc, :TCH], Ls[qb:qb + 64, :],
                            qT[qb:qb + 64, h // 2, ncc * TCH:(ncc + 1) * TCH],
                            start=True, stop=True,
                        )
                    ys = ysq_pool.tile([HD, TOK], bf16, tag="ys")
                    nc.scalar.activation(
                        r4(ys[:]), yp[:, :, :TCH], AF.Square,
                        accum_out=ysq[:, h:h + 1],
                    )

            # ============== Phase S: fold stats, AllReduce ===============
            with tc.tile_pool(name="pss", bufs=1, space="PSUM") as spool:
                psA = spool.tile([1, 512], f32, tag="psA")
                nc.tensor.matmul(psA[:, 0:6], slhsA[:, 0:1], qcol[:], start=True, stop=True)
                nc.tensor.matmul(psA[:, 8:14], slhsA[:, 1:2], qcol[:], start=True, stop=True)
                nc.tensor.matmul(psA[:, 32:44], ones64[:], ysq[:], start=True, stop=True)
                nc.vector.tensor_copy(AR[0:1, 0:12:2], psA[:, 0:6])
                nc.vector.tensor_copy(AR[0:1, 1:12:2], psA[:, 8:14])
                nc.vector.tensor_copy(AR[0:1, 12:24], psA[:, 32:44])

            nc.sync.dma_start(arin[:], AR[:])
            nc.gpsimd.collective_compute(
                "AllReduce", OP.add,
                ins=[arin.opt()], outs=[arout.opt()],
                replica_groups=[list(range(8))],
            )
            nc.sync.dma_start(Sg[:], arout[:])

            # ============== Phase P: BN affine params ====================
            # layout of prm scratch: each slot [1, 12]
            def ps(i):
                return prm[:, i * 12:(i + 1) * 12]

            mean_a, ex2_a, var_a, rstd_a, alpha_a, expsc = (ps(i) for i in range(6))
            mean_v, ex2_v, var_v, rstd_v, alpha_v, cv = (ps(i) for i in range(6, 12))

            with tc.tile_pool(name="psp", bufs=1, space="PSUM") as bpool:
                nc.vector.tensor_scalar_mul(mean_a, Sg[:, 0:12], 1.0 / NA)
                nc.vector.tensor_scalar_mul(ex2_a, Sg[:, 12:24], 1.0 / NA)
                nc.vector.tensor_tensor(var_a, mean_a, mean_a, OP.mult)
                nc.vector.tensor_sub(var_a, ex2_a, var_a)
                nc.vector.tensor_scalar_add(var_a, var_a, BN_EPS)
                nc.scalar.activation(rstd_a, var_a, AF.Sqrt)
                nc.vector.reciprocal(rstd_a, rstd_a)
                nc.vector.tensor_tensor(alpha_a, gam[:], rstd_a, OP.mult)
                nc.vector.tensor_scalar_mul(expsc, alpha_a, SCALE)

                nc.vector.tensor_scalar_mul(mean_v, Sg[:, 24:36], 1.0 / NV)
                nc.vector.tensor_scalar_mul(ex2_v, Sg[:, 36:48], 1.0 / NV)
                nc.vector.tensor_tensor(var_v, mean_v, mean_v, OP.mult)
                nc.vector.tensor_sub(var_v, ex2_v, var_v)
                nc.vector.tensor_scalar_add(var_v, var_v, BN_EPS)
                nc.scalar.activation(rstd_v, var_v, AF.Sqrt)
                nc.vector.reciprocal(rstd_v, rstd_v)
                nc.vector.tensor_tensor(alpha_v, gam[:], rstd_v, OP.mult)
                nc.vector.tensor_tensor(cv, mean_v, alpha_v, OP.mult)
                nc.vector.tensor_sub(cv, bet[:], cv)

                nc.gpsimd.partition_broadcast(expscb[:], expsc)
                nc.gpsimd.partition_broadcast(avb[:], alpha_v)
                nc.vector.tensor_copy(avc[0:64, :], avb[0:64, 0:12:2])
                nc.vector.tensor_copy(avc[64:128, :], avb[64:128, 1:12:2])
                for t in range(6):
                    nc.vector.tensor_scalar_mul(
                        weff[:, t, :], wp[:, t, :], avc[:, t:t + 1]
                    )
                # b_eff = c_v @ W_proj + b_proj  (via R = head-rowsums of W_proj)
                # 12-elem transpose via DRAM bounce (SBUF APs can't swap axes)
                nc.sync.dma_start(cvd[:].rearrange("h a -> a h"), cv)
                nc.sync.dma_start(cvT[:], cvd[:])
                bep = bpool.tile([1, 2, 512], f32, tag="bep")
                for n2 in range(2):
                    nc.tensor.matmul(
                        bep[:, n2, :384], cvT[:], Rs[:, n2 * 384:(n2 + 1) * 384],
                        start=True, stop=True,
                    )
                nc.vector.tensor_tensor(
                    r2(beffr[:]), bep[:, :, :384],
                    r2(bproj[:]), OP.add,
                )
                nc.gpsimd.partition_broadcast(beffb[:], beffr[:])

            # ============== Phase A: scores/softmax/attn@v per head ======
            with (
                tc.tile_pool(name="pssc", bufs=1, space="PSUM") as scpool,
                tc.tile_pool(name="psop", bufs=1, space="PSUM") as opool,
                tc.tile_pool(name="expp", bufs=2) as expool,
                tc.tile_pool(name="rsp", bufs=2) as rspool,
                tc.tile_pool(name="rbp", bufs=2) as rbpool,
            ):
                for h in range(H):
                    qb = (h % 2) * 64
                    expt = expool.tile([PC, 2, TOK], bf16, tag="exp")
                    for pc in range(2):
                        sp = scpool.tile([PC, 4, 512], f32, tag="sc")
                        for ncc in range(4):
                            nc.tensor.matmul(
                                sp[:, ncc, :TCH],
                                kT[qb:qb + 64, pc * PC:(pc + 1) * PC],
                                qT[qb:qb + 64, h // 2, ncc * TCH:(ncc + 1) * TCH],
                                start=True, stop=True,
                            )
                        nc.scalar.activation(
                            r4(expt[:, pc, :]), sp[:, :, :TCH], AF.Exp,
                            bias=sbias[:, pc:pc + 1],
                            scale=expscb[0:PC, h:h + 1],
                        )
                    op = opool.tile([HD + 1, 8, 256], f32, tag="op")
                    for b in range(BL):
                        for pc in range(2):
                            nc.tensor.matmul(
                                op[:, b, :N],
                                vpr[:, 2 * b + pc, h, :],
                                expt[:, pc, b * N:(b + 1) * N],
                                start=(pc == 0), stop=(pc == 1),
                            )
                    rsr = rspool.tile([1, TOK], f32, tag="rsr")
                    nc.vector.reciprocal(r8(rsr[:]), op[HD:HD + 1, :, :N])
                    rb = rbpool.tile([HD, TOK], f32, tag="rb")
                    nc.gpsimd.partition_broadcast(rb[:], rsr[:])
                    nc.vector.tensor_tensor(
                        r8(U_T[qb:qb + 64, h // 2, :]),
                        op[0:HD, :, :N],
                        r8(rb[:]),
                        OP.mult,
                    )

            # ============== Phase O: projection + bias ===================
            with (
                tc.tile_pool(name="psp2", bufs=2, space="PSUM") as ppool,
                tc.tile_pool(name="ostp", bufs=3) as ostp,
            ):
                for m in range(13):
                    rows = 128 if m < 12 else 32
                    pmm = ppool.tile([128, 2, 512], f32, tag="pmm")
                    for n2 in range(2):
                        for kc in range(6):
                            nc.tensor.matmul(
                                pmm[:rows, n2, :384],
                                U_T[:, kc, m * 128:m * 128 + rows],
                                weff[:, kc, n2 * 384:(n2 + 1) * 384],
                                start=(kc == 0), stop=(kc == 5),
                            )
                    ost = ostp.tile([128, C], f32, tag="ost")
                    nc.vector.tensor_tensor(
                        r2(ost[:rows, :]), pmm[:rows, :, :384],
                        r2(beffb[:rows, :]), OP.add,
                    )
                    nc.sync.dma_start(out_d.ap()[m * 128:m * 128 + rows, :], ost[:rows, :])

    nc.compile()
    return nc


def _get_nc():
    if "nc" not in _NC_CACHE:
        _NC_CACHE["nc"] = _build_nc()
    return _NC_CACHE["nc"]


def _host_prep(inputs):
    x = np.asarray(inputs["x"], np.float32)
    W_qv = np.asarray(inputs["W_qv"], np.float32)
    k_ext = np.asarray(inputs["k_ext"], np.float32)
    attn_bias = np.asarray(inputs["attn_bias"], np.float32).reshape(1, N)
    gamma = np.asarray(inputs["bn_gamma"], np.float32).reshape(1, H)
    beta = np.asarray(inputs["bn_beta"], np.float32).reshape(1, H)
    W_proj = np.asarray(inputs["W_proj"], np.float32)
    b_proj = np.asarray(inputs["b_proj"], np.float32).reshape(1, C)

    wq_bf = np.ascontiguousarray(W_qv[:, :C]).astype(BF)
    wv_bf = np.ascontiguousarray(W_qv[:, C:]).astype(BF)
    wp_bf = W_proj.astype(BF)
    kT1 = np.ascontiguousarray(k_ext.T).astype(BF)
    kT_bf = np.concatenate([kT1, kT1], axis=0)  # duplicated in both halves

    G = k_ext.astype(np.float64)
    G = G.T @ G
    L = np.linalg.cholesky(G + 1e-6 * np.eye(HD)).astype(np.float32)
    L2 = np.concatenate([L, L], axis=0)  # duplicated in both halves

    ksum = k_ext.sum(0).astype(np.float32)
    slhsA = np.zeros((128, 2), np.float32)
    slhsA[0:64, 0] = ksum
    slhsA[64:128, 1] = ksum

    sbias = np.ascontiguousarray(
        (SCALE * attn_bias.reshape(2, PC)).T
    ).astype(np.float32)

    R = W_proj.reshape(H, HD, C).sum(1).astype(np.float32)

    common = dict(
        wq=wq_bf, wv=wv_bf, wp=wp_bf, kT=kT_bf, L=L2.astype(BF),
        slhsA=slhsA, sbias=sbias, gamma=gamma, beta=beta,
        bproj=b_proj, R=R,
    )
    in_maps = []
    for c in range(8):
        xs = x[c * BL:(c + 1) * BL].reshape(TOK, C)
        xT = np.ascontiguousarray(xs.T).astype(BF)
        in_maps.append(dict(common, xT=xT))
    return in_maps


def kernel(**inputs):
    from concourse.bass_utils import run_bass_kernel_spmd

    in_maps = _host_prep(inputs)
    nc = _get_nc()
    res = run_bass_kernel_spmd(nc, in_maps, core_ids=list(range(8)))
    outs = [res.results[c]["out"].reshape(BL, N, C) for c in range(8)]
    return np.concatenate(outs, axis=0)
